# revision 9
# baseline (speedup 1.0000x reference)
"""Trainium2 Bass kernel for nn_PolicyNetwork (encoder/decoder LSTM + attention).

Math (per batch b):
  enc_h[t]  = LSTM_enc(x[t]) recurrence over T steps
  dec_h[t]  = LSTM_dec(x_dec[t]) recurrence, x_dec[t] = x[idx[t-1]] (host gather)
  S[t,t']   = dec_h[t] . enc_h[t'];  attn = softmax(S);  ctx = attn @ enc_h
  logits    = ctx @ Wd.T + bd  (then -inf mask applied host-side)
  rot       = ctx @ Wr.T + br

Sharding: data-parallel over batch, B=512 -> 64 per core on 8 cores.
On-device layout: the two recurrences run interleaved; combined state is
[2*bl, H] with enc batch on partitions 0:bl and dec batch on bl:2*bl.  The
recurrent gate matmuls use PE column tiling (tile_position (0,0)/(0,bl)) so
both halves stream concurrently through the 128x128 array.
"""

import numpy as np
from contextlib import ExitStack

import concourse.bass as bass
import concourse.mybir as mybir
import concourse.tile as tile
from concourse.bass_utils import run_bass_kernel_spmd

B, T, D, H = 512, 128, 256, 512
L = T
N_CORES = 8
BL = B // N_CORES          # 64 batch per core
G4 = 4 * H                 # 2048 gates
KD = D // 128              # 2 contraction tiles for input proj
KH = H // 128              # 4 contraction tiles for recurrent proj
# process order: g first (tanh, feeds c), then i, f, o (torch order i,f,g,o)
CHUNK_ORDER = (2, 0, 1, 3)
FP = mybir.dt.float32
AF = mybir.ActivationFunctionType

_BUILD_CACHE: dict = {}

# set by test harnesses to capture a profile; harmless defaults for grading
TRACE = False
TRACE_DIR = None
LAST_EXEC_NS = None


def _split_sync_waits(nc, maxw=1):
    """This container's walrus only accepts one sync-wait per instruction;
    hoist excess waits onto inserted NOPs on the same engine."""
    for fn in nc.m.functions:
        for bb in fn.blocks:
            out = []
            changed = False
            for inst in list(bb.instructions):
                si = inst.sync_info
                if (
                    si is not None
                    and len(si.on_wait) > maxw
                    and "Dma" not in type(inst).__name__
                ):
                    waits = list(si.on_wait)
                    keep, extra = waits[-maxw:], waits[:-maxw]
                    while extra:
                        chunk, extra = extra[:maxw], extra[maxw:]
                        out.append(
                            mybir.InstNoOp(
                                name=nc.get_next_instruction_name(),
                                sync_info=mybir.SyncInfo(on_wait=chunk, on_update=[]),
                                bass_nofuse=True,
                                engine=inst.engine,
                            )
                        )
                    inst.sync_info = mybir.SyncInfo(
                        on_wait=keep, on_update=list(si.on_update)
                    )
                    changed = True
                out.append(inst)
            if changed:
                bb.instructions = out
    return nc


def build_nc(t_steps=T, bl=BL, gate_bias=False, out_bias=False, use_coltile=True):
    """Build the SPMD Bass program for one core (bl batch, t_steps time)."""
    assert bl % 32 == 0 and 2 * bl <= 128
    nc = bass.Bass()
    TS = t_steps               # logits dim == seq len
    PB = 2 * bl                # combined enc+dec partitions
    DO = bl                    # decoder partition/column-group offset

    xte = nc.dram_tensor("xt_enc", [128, KD, TS, bl], FP, kind="ExternalInput")
    xtd = nc.dram_tensor("xt_dec", [128, KD, TS, bl], FP, kind="ExternalInput")
    wih_e = nc.dram_tensor("wih_enc", [128, KD, G4], FP, kind="ExternalInput")
    whh_e = nc.dram_tensor("whh_enc", [128, KH, G4], FP, kind="ExternalInput")
    wih_d = nc.dram_tensor("wih_dec", [128, KD, G4], FP, kind="ExternalInput")
    whh_d = nc.dram_tensor("whh_dec", [128, KH, G4], FP, kind="ExternalInput")
    wd = nc.dram_tensor("wdT", [128, KH, TS], FP, kind="ExternalInput")
    wr = nc.dram_tensor("wrT", [128, KH, 4], FP, kind="ExternalInput")
    ident = nc.dram_tensor("ident", [128, 128], FP, kind="ExternalInput")
    if gate_bias:
        gb_e = nc.dram_tensor("gbias_enc", [1, G4], FP, kind="ExternalInput")
        gb_d = nc.dram_tensor("gbias_dec", [1, G4], FP, kind="ExternalInput")
    if out_bias:
        bd_t = nc.dram_tensor("bd", [1, TS], FP, kind="ExternalInput")
        br_t = nc.dram_tensor("br", [1, 4], FP, kind="ExternalInput")
    out_logits = nc.dram_tensor("logits", [bl, TS, TS], FP, kind="ExternalOutput")
    out_rot = nc.dram_tensor("rot", [bl, TS, 4], FP, kind="ExternalOutput")

    def mm(ps, lhsT, rhs, start, stop, col):
        nc.tensor.matmul(
            ps, lhsT, rhs, start=start, stop=stop,
            tile_position=(0, DO * col) if use_coltile else None,
        )

    with tile.TileContext(nc) as tc, ExitStack() as ctx:
        wpool = ctx.enter_context(tc.tile_pool(name="weights", bufs=1))
        dram = ctx.enter_context(tc.tile_pool(name="dram", bufs=1, space="DRAM"))

        def load_w(src, shape):
            t = wpool.tile(shape, FP, tag=src.name)
            nc.sync.dma_start(t[:], src[:])
            return t

        wih_e_sb = load_w(wih_e, [128, KD, G4])
        whh_e_sb = load_w(whh_e, [128, KH, G4])
        wih_d_sb = load_w(wih_d, [128, KD, G4])
        whh_d_sb = load_w(whh_d, [128, KH, G4])
        wd_sb = load_w(wd, [128, KH, TS])
        wr_sb = load_w(wr, [128, KH, 4])
        ident_sb = load_w(ident, [128, 128])
        if gate_bias:
            gb_e_sb = load_w(gb_e, [1, G4])
            gb_d_sb = load_w(gb_d, [1, G4])
        if out_bias:
            bd_sb = load_w(bd_t, [1, TS])
            br_sb = load_w(br_t, [1, 4])
        if gate_bias or out_bias:
            ones_sb = wpool.tile([1, 128], FP, tag="ones")
            nc.vector.memset(ones_sb[:], 1.0)

        encP = dram.tile([bl, TS, H], FP, tag="encP")
        decP = dram.tile([bl, TS, H], FP, tag="decP")

        state = ctx.enter_context(tc.tile_pool(name="state", bufs=1))
        c_sb = state.tile([PB, H], FP, tag="c")
        nc.vector.memset(c_sb[:], 0.0)
        hT_sb = state.tile([128, KH * PB], FP, tag="hT")
        nc.vector.memset(hT_sb[:], 0.0)

        # ---------------- phase 1: interleaved LSTM recurrences ----------------
        with (
            tc.tile_pool(name="xin", bufs=4) as xpool,
            tc.tile_pool(name="gates_ps", bufs=6, space="PSUM") as gps,
            tc.tile_pool(name="hT_ps", bufs=2, space="PSUM") as hTps,
            tc.tile_pool(name="acts", bufs=2) as acts,
            tc.tile_pool(name="elem", bufs=2) as elem,
        ):
            for t in range(t_steps):
                xe = xpool.tile([128, KD, bl], FP, tag="xe")
                nc.sync.dma_start(xe[:], xte[:, :, t, :])
                xd = xpool.tile([128, KD, bl], FP, tag="xd")
                nc.sync.dma_start(xd[:], xtd[:, :, t, :])

                gate_sb = {}
                for n in CHUNK_ORDER:
                    sl = slice(512 * n, 512 * (n + 1))
                    ps = gps.tile([PB, 512], FP, tag="gates")
                    for kt in range(KD):
                        mm(ps[0:bl, :], xe[:, kt, :], wih_e_sb[:, kt, sl],
                           kt == 0, False, 0)
                        mm(ps[DO:DO + bl, :], xd[:, kt, :],
                           wih_d_sb[:, kt, sl], kt == 0, False, 1)
                    for kt in range(KH):
                        last = (kt == KH - 1) and not gate_bias
                        mm(ps[0:bl, :], hT_sb[:, PB * kt:PB * kt + bl],
                           whh_e_sb[:, kt, sl], False, last, 0)
                        mm(ps[DO:DO + bl, :],
                           hT_sb[:, PB * kt + DO:PB * kt + DO + bl],
                           whh_d_sb[:, kt, sl], False, last, 1)
                    if gate_bias:
                        mm(ps[0:bl, :], ones_sb[0:1, 0:bl], gb_e_sb[0:1, sl],
                           False, True, 0)
                        mm(ps[DO:DO + bl, :], ones_sb[0:1, 0:bl],
                           gb_d_sb[0:1, sl], False, True, 1)
                    a_sb = acts.tile([PB, 512], FP, tag=f"act{n}")
                    func = AF.Tanh if n == 2 else AF.Sigmoid
                    nc.scalar.activation(a_sb[:], ps[:], func)
                    gate_sb[n] = a_sb

                t2 = elem.tile([PB, H], FP, tag="t2")
                nc.vector.tensor_mul(t2[:], gate_sb[0][:], gate_sb[2][:])  # i'*g'
                t1 = elem.tile([PB, H], FP, tag="t1")
                nc.vector.tensor_mul(t1[:], gate_sb[1][:], c_sb[:])        # f'*c
                nc.vector.tensor_add(c_sb[:], t1[:], t2[:])
                tnc = elem.tile([PB, H], FP, tag="tanh_c")
                nc.scalar.activation(tnc[:], c_sb[:], AF.Tanh)
                h_sb = elem.tile([PB, H], FP, tag="h")
                nc.vector.tensor_mul(h_sb[:], gate_sb[3][:], tnc[:])       # o'*tanh(c)

                nc.sync.dma_start(encP[:, t, :], h_sb[0:bl, :])
                nc.sync.dma_start(decP[:, t, :], h_sb[DO:DO + bl, :])

                hT_p = hTps.tile([128, KH * PB], FP, tag="hTp")
                for k in range(KH):
                    nc.tensor.transpose(
                        hT_p[:, PB * k:PB * (k + 1)],
                        h_sb[:, 128 * k:128 * (k + 1)], ident_sb[0:PB, 0:PB])
                nc.vector.tensor_copy(hT_sb[:], hT_p[:])

        # ---------------- phase 2: attention + output heads --------------------
        with (
            tc.tile_pool(name="pin", bufs=3) as pin,
            tc.tile_pool(name="ptmp", bufs=3) as ptmp,
            tc.tile_pool(name="psm", bufs=3) as psm,
            tc.tile_pool(name="tr_ps", bufs=1, space="PSUM") as trps,
            tc.tile_pool(name="s_ps", bufs=1, space="PSUM") as sps,
            tc.tile_pool(name="at_ps", bufs=1, space="PSUM") as atps,
            tc.tile_pool(name="cx_ps", bufs=1, space="PSUM") as cxps,
            tc.tile_pool(name="l_ps", bufs=1, space="PSUM") as lps,
            tc.tile_pool(name="r_ps", bufs=1, space="PSUM") as rps,
        ):
            for b in range(bl):
                encp_t = pin.tile([TS, H], FP, tag="encp")
                nc.sync.dma_start(encp_t[:], encP[b, :, :])
                decp_t = pin.tile([TS, H], FP, tag="decp")
                nc.sync.dma_start(decp_t[:], decP[b, :, :])

                trd = trps.tile([128, KH * TS], FP, tag="trd")
                for k in range(KH):
                    nc.tensor.transpose(
                        trd[:, TS * k:TS * (k + 1)],
                        decp_t[:, 128 * k:128 * (k + 1)], ident_sb[0:TS, 0:TS])
                decT = ptmp.tile([128, KH * TS], FP, tag="decT")
                nc.vector.tensor_copy(decT[:], trd[:])

                tre = trps.tile([128, KH * TS], FP, tag="tre")
                for k in range(KH):
                    nc.tensor.transpose(
                        tre[:, TS * k:TS * (k + 1)],
                        encp_t[:, 128 * k:128 * (k + 1)], ident_sb[0:TS, 0:TS])
                encT = ptmp.tile([128, KH * TS], FP, tag="encT")
                nc.scalar.copy(encT[:], tre[:])

                s_ps = sps.tile([TS, TS], FP, tag="s")
                for k in range(KH):
                    nc.tensor.matmul(
                        s_ps[:], decT[:, TS * k:TS * (k + 1)],
                        encT[:, TS * k:TS * (k + 1)],
                        start=(k == 0), stop=(k == KH - 1))

                negmax = psm.tile([TS, 1], FP, tag="negmax")
                nc.vector.reduce_max(
                    negmax[:], s_ps[:], axis=mybir.AxisListType.X, negate=True)
                esb = psm.tile([TS, TS], FP, tag="esb")
                rowsum = psm.tile([TS, 1], FP, tag="rowsum")
                nc.scalar.activation(
                    esb[:], s_ps[:], AF.Exp, bias=negmax[:], scale=1.0,
                    accum_out=rowsum[:])
                recip = psm.tile([TS, 1], FP, tag="recip")
                nc.vector.reciprocal(recip[:], rowsum[:])
                attn = psm.tile([TS, TS], FP, tag="attn")
                nc.vector.tensor_scalar_mul(attn[:], esb[:], recip[:])

                at_ps = atps.tile([TS, TS], FP, tag="at")
                nc.tensor.transpose(at_ps[:], attn[:], ident_sb[0:TS, 0:TS])
                atT = psm.tile([TS, TS], FP, tag="atT")
                nc.vector.tensor_copy(atT[:], at_ps[:])

                cx_ps = cxps.tile([128, KH * TS], FP, tag="cx")
                for m in range(KH):
                    nc.tensor.matmul(
                        cx_ps[:, TS * m:TS * (m + 1)],
                        encp_t[:, 128 * m:128 * (m + 1)], atT[:],
                        start=True, stop=True)
                ctxT = ptmp.tile([128, KH * TS], FP, tag="ctxT")
                nc.scalar.copy(ctxT[:], cx_ps[:])

                l_ps = lps.tile([TS, TS], FP, tag="l")
                for k in range(KH):
                    nc.tensor.matmul(
                        l_ps[:], ctxT[:, TS * k:TS * (k + 1)],
                        wd_sb[:, k, :], start=(k == 0),
                        stop=(k == KH - 1) and not out_bias)
                if out_bias:
                    nc.tensor.matmul(l_ps[:], ones_sb[0:1, 0:TS],
                                     bd_sb[0:1, :], start=False, stop=True)
                l_sb = psm.tile([TS, TS], FP, tag="l_sb")
                nc.scalar.copy(l_sb[:], l_ps[:])
                nc.sync.dma_start(out_logits[b, :, :], l_sb[:])

                r_ps = rps.tile([TS, 4], FP, tag="r")
                for k in range(KH):
                    nc.tensor.matmul(
                        r_ps[:], ctxT[:, TS * k:TS * (k + 1)],
                        wr_sb[:, k, :], start=(k == 0),
                        stop=(k == KH - 1) and not out_bias)
                if out_bias:
                    nc.tensor.matmul(r_ps[:], ones_sb[0:1, 0:TS],
                                     br_sb[0:1, :], start=False, stop=True)
                r_sb = psm.tile([TS, 4], FP, tag="r_sb")
                nc.vector.tensor_copy(r_sb[:], r_ps[:])
                nc.sync.dma_start(out_rot[b, :, :], r_sb[:])

    return _split_sync_waits(nc)


def pack_x(xs, t_steps, bl):
    # [bl, t, D] -> [128, KD, t, bl] with d = k*128 + p
    return np.ascontiguousarray(
        xs.transpose(2, 1, 0).reshape(KD, 128, t_steps, bl).transpose(1, 0, 2, 3)
    ).astype(np.float32)


def pack_w(w, kt):
    # [M, K] weight -> rhs-stream layout [128, kt, M] with k = ki*128 + p
    wT = np.ascontiguousarray(np.asarray(w).T)  # [K, M]
    return np.ascontiguousarray(
        wT.reshape(kt, 128, w.shape[0]).transpose(1, 0, 2)
    ).astype(np.float32)


def _get_nc(key):
    if key not in _BUILD_CACHE:
        gate_bias, out_bias = key
        _BUILD_CACHE[key] = build_nc(gate_bias=gate_bias, out_bias=out_bias)
    return _BUILD_CACHE[key]


def kernel(inputs, target_indices, enc_Wih, enc_Whh, enc_bih, enc_bhh,
           dec_Wih, dec_Whh, dec_bih, dec_bhh, Wd, bd, Wr, br):
    x = np.asarray(inputs, dtype=np.float32)
    idx = np.asarray(target_indices)
    idxc = np.clip(idx.astype(np.int64), 0, T - 1)

    # decoder inputs: x_dec[:, 0] = 0, x_dec[:, t] = x[b, idx[b, t-1]]
    xdec = np.zeros_like(x)
    xdec[:, 1:, :] = np.take_along_axis(x, idxc[:, :-1, None], axis=1)

    gbias_e = (np.asarray(enc_bih) + np.asarray(enc_bhh)).astype(np.float32)
    gbias_d = (np.asarray(dec_bih) + np.asarray(dec_bhh)).astype(np.float32)
    bd_np = np.asarray(bd, dtype=np.float32)
    br_np = np.asarray(br, dtype=np.float32)
    gate_bias = bool(np.any(gbias_e) or np.any(gbias_d))
    out_bias = bool(np.any(bd_np) or np.any(br_np))

    nc = _get_nc((gate_bias, out_bias))

    shared = {
        "wih_enc": pack_w(np.asarray(enc_Wih), KD),
        "whh_enc": pack_w(np.asarray(enc_Whh), KH),
        "wih_dec": pack_w(np.asarray(dec_Wih), KD),
        "whh_dec": pack_w(np.asarray(dec_Whh), KH),
        "wdT": pack_w(np.asarray(Wd), KH),
        "wrT": pack_w(np.asarray(Wr), KH),
        "ident": np.eye(128, dtype=np.float32),
    }
    if gate_bias:
        shared["gbias_enc"] = gbias_e[None, :]
        shared["gbias_dec"] = gbias_d[None, :]
    if out_bias:
        shared["bd"] = bd_np[None, :]
        shared["br"] = br_np[None, :]

    in_maps = []
    for c in range(N_CORES):
        b0 = c * BL
        m = dict(shared)
        m["xt_enc"] = pack_x(x[b0:b0 + BL], T, BL)
        m["xt_dec"] = pack_x(xdec[b0:b0 + BL], T, BL)
        in_maps.append(m)

    res = run_bass_kernel_spmd(
        nc, in_maps, list(range(N_CORES)), trace=TRACE,
        tmpdir=TRACE_DIR if TRACE else None)
    global LAST_EXEC_NS
    LAST_EXEC_NS = res.exec_time_ns
    logits = np.concatenate(
        [res.results[c]["logits"] for c in range(N_CORES)], axis=0)
    rot = np.concatenate(
        [res.results[c]["rot"] for c in range(N_CORES)], axis=0)

    # -inf mask: position l masked at step t if l in {idx[0..t-1]}
    onehot = np.zeros((B, T, L), dtype=bool)
    onehot[np.arange(B)[:, None], np.arange(T)[None, :], idxc] = True
    mask = np.zeros((B, T, L), dtype=bool)
    mask[:, 1:, :] = np.cumsum(onehot[:, :-1, :], axis=1) > 0
    logits[mask] = -np.inf
    return logits, rot


# revision 15
# speedup vs baseline: 1.6676x; 1.6676x over previous
"""Trainium2 Bass kernel for nn_PolicyNetwork (encoder/decoder LSTM + attention).

Math (per batch b):
  enc_h[t]  = LSTM_enc(x[t]) recurrence over T steps
  dec_h[t]  = LSTM_dec(x_dec[t]) recurrence, x_dec[t] = x[idx[t-1]] (host gather)
  S[t,t']   = dec_h[t] . enc_h[t'];  attn = softmax(S);  ctx = attn @ enc_h
  logits    = ctx @ Wd.T + bd  (then -inf mask applied host-side)
  rot       = ctx @ Wr.T + br

Sharding: data-parallel over batch, B=512 -> 64 per core on 8 cores.
On-device layout: the two recurrences run interleaved; combined state is
[2*bl, H] with enc batch on partitions 0:bl and dec batch on bl:2*bl.  The
recurrent gate matmuls use PE column tiling (tile_position (0,0)/(0,bl)) so
both halves stream concurrently through the 128x128 array.

All matmul operands are float32r (single-pass PE mode, 4x the fp32 rate at
moving dim >= 256); PSUM accumulation stays fp32, the cell state c stays
fp32.
"""

import numpy as np
from contextlib import ExitStack

import concourse.bass as bass
import concourse.mybir as mybir
import concourse.tile as tile
from concourse.bass_utils import run_bass_kernel_spmd

B, T, D, H = 512, 128, 256, 512
L = T
N_CORES = 8
BL = B // N_CORES          # 64 batch per core
G4 = 4 * H                 # 2048 gates
KD = D // 128              # 2 contraction tiles for input proj
KH = H // 128              # 4 contraction tiles for recurrent proj
# process order: g first (tanh, feeds c), then i, f, o (torch order i,f,g,o)
CHUNK_ORDER = (2, 0, 1, 3)
FP = mybir.dt.float32
AF = mybir.ActivationFunctionType

_BUILD_CACHE: dict = {}

# set by test harnesses to capture a profile; harmless defaults for grading
TRACE = False
TRACE_DIR = None
LAST_EXEC_NS = None


def _split_sync_waits(nc, maxw=1):
    """This container's walrus only accepts one sync-wait per instruction;
    hoist excess waits onto inserted NOPs on the same engine."""
    for fn in nc.m.functions:
        for bb in fn.blocks:
            out = []
            changed = False
            for inst in list(bb.instructions):
                si = inst.sync_info
                if (
                    si is not None
                    and len(si.on_wait) > maxw
                    and "Dma" not in type(inst).__name__
                ):
                    waits = list(si.on_wait)
                    keep, extra = waits[-maxw:], waits[:-maxw]
                    while extra:
                        chunk, extra = extra[:maxw], extra[maxw:]
                        out.append(
                            mybir.InstNoOp(
                                name=nc.get_next_instruction_name(),
                                sync_info=mybir.SyncInfo(on_wait=chunk, on_update=[]),
                                bass_nofuse=True,
                                engine=inst.engine,
                            )
                        )
                    inst.sync_info = mybir.SyncInfo(
                        on_wait=keep, on_update=list(si.on_update)
                    )
                    changed = True
                out.append(inst)
            if changed:
                bb.instructions = out
    return nc


def build_nc(t_steps=T, bl=BL, gate_bias=False, out_bias=False, use_coltile=True,
             f32r=True):
    """Build the SPMD Bass program for one core (bl batch, t_steps time)."""
    assert bl % 32 == 0 and 2 * bl <= 128
    nc = bass.Bass()
    TS = t_steps               # logits dim == seq len
    PB = 2 * bl                # combined enc+dec partitions
    DO = bl                    # decoder partition/column-group offset
    FR = mybir.dt.float32r if f32r else FP

    xte = nc.dram_tensor("xt_enc", [128, KD, TS, bl], FR, kind="ExternalInput")
    xtd = nc.dram_tensor("xt_dec", [128, KD, TS, bl], FR, kind="ExternalInput")
    wih_e = nc.dram_tensor("wih_enc", [128, KD, G4], FR, kind="ExternalInput")
    whh_e = nc.dram_tensor("whh_enc", [128, KH, G4], FR, kind="ExternalInput")
    wih_d = nc.dram_tensor("wih_dec", [128, KD, G4], FR, kind="ExternalInput")
    whh_d = nc.dram_tensor("whh_dec", [128, KH, G4], FR, kind="ExternalInput")
    wd = nc.dram_tensor("wdT", [128, KH, TS], FR, kind="ExternalInput")
    wr = nc.dram_tensor("wrT", [128, KH, 4], FR, kind="ExternalInput")
    ident = nc.dram_tensor("ident", [128, 128], FR, kind="ExternalInput")
    if gate_bias:
        gb_e = nc.dram_tensor("gbias_enc", [1, G4], FR, kind="ExternalInput")
        gb_d = nc.dram_tensor("gbias_dec", [1, G4], FR, kind="ExternalInput")
    if out_bias:
        bd_t = nc.dram_tensor("bd", [1, TS], FR, kind="ExternalInput")
        br_t = nc.dram_tensor("br", [1, 4], FR, kind="ExternalInput")
    out_logits = nc.dram_tensor("logits", [bl, TS, TS], FP, kind="ExternalOutput")
    out_rot = nc.dram_tensor("rot", [bl, TS, 4], FP, kind="ExternalOutput")

    def mm(ps, lhsT, rhs, start, stop, col):
        # fp32r does not support PE column tiling; enc/dec run sequentially
        # into separate PSUM tiles, both at partition base 0.
        nc.tensor.matmul(ps, lhsT, rhs, start=start, stop=stop)

    with tile.TileContext(nc) as tc, ExitStack() as ctx:
        wpool = ctx.enter_context(tc.tile_pool(name="weights", bufs=1))
        dram = ctx.enter_context(tc.tile_pool(name="dram", bufs=1, space="DRAM"))

        def load_w(src, shape):
            t = wpool.tile(shape, FR, tag=src.name)
            nc.sync.dma_start(t[:], src[:])
            return t

        wih_e_sb = load_w(wih_e, [128, KD, G4])
        whh_e_sb = load_w(whh_e, [128, KH, G4])
        wih_d_sb = load_w(wih_d, [128, KD, G4])
        whh_d_sb = load_w(whh_d, [128, KH, G4])
        wd_sb = load_w(wd, [128, KH, TS])
        wr_sb = load_w(wr, [128, KH, 4])
        ident_sb = load_w(ident, [128, 128])
        if gate_bias:
            gb_e_sb = load_w(gb_e, [1, G4])
            gb_d_sb = load_w(gb_d, [1, G4])
        if out_bias:
            bd_sb = load_w(bd_t, [1, TS])
            br_sb = load_w(br_t, [1, 4])
        if gate_bias or out_bias:
            ones_sb = wpool.tile([1, 128], FR, tag="ones")
            nc.vector.memset(ones_sb[:].bitcast(FP), 1.0)

        encP = dram.tile([bl, TS, H], FR, tag="encP")
        decP = dram.tile([bl, TS, H], FR, tag="decP")

        state = ctx.enter_context(tc.tile_pool(name="state", bufs=1))
        c_sb = state.tile([PB, H], FP, tag="c")
        nc.vector.memset(c_sb[:], 0.0)
        hT_sb = state.tile([128, KH * PB], FR, tag="hT")
        nc.vector.memset(hT_sb[:].bitcast(FP), 0.0)

        # ---------------- phase 1: interleaved LSTM recurrences ----------------
        with (
            tc.tile_pool(name="xin", bufs=4) as xpool,
            tc.tile_pool(name="gates_ps", bufs=3, space="PSUM") as gps,
            tc.tile_pool(name="hT_ps", bufs=2, space="PSUM") as hTps,
            tc.tile_pool(name="acts", bufs=2) as acts,
            tc.tile_pool(name="elem", bufs=2) as elem,
        ):
            for t in range(t_steps):
                xe = xpool.tile([128, KD, bl], FR, tag="xe")
                nc.sync.dma_start(xe[:], xte[:, :, t, :])
                xd = xpool.tile([128, KD, bl], FR, tag="xd")
                nc.sync.dma_start(xd[:], xtd[:, :, t, :])

                gate_sb = {}
                for n in CHUNK_ORDER:
                    sl = slice(512 * n, 512 * (n + 1))
                    ps_e = gps.tile([bl, 512], FP, tag="gates_e")
                    ps_d = gps.tile([bl, 512], FP, tag="gates_d")
                    for kt in range(KD):
                        mm(ps_e[:, :], xe[:, kt, :], wih_e_sb[:, kt, sl],
                           kt == 0, False, 0)
                        mm(ps_d[:, :], xd[:, kt, :],
                           wih_d_sb[:, kt, sl], kt == 0, False, 1)
                    for kt in range(KH):
                        last = (kt == KH - 1) and not gate_bias
                        mm(ps_e[:, :], hT_sb[:, PB * kt:PB * kt + bl],
                           whh_e_sb[:, kt, sl], False, last, 0)
                        mm(ps_d[:, :],
                           hT_sb[:, PB * kt + DO:PB * kt + DO + bl],
                           whh_d_sb[:, kt, sl], False, last, 1)
                    if gate_bias:
                        mm(ps_e[:, :], ones_sb[0:1, 0:bl], gb_e_sb[0:1, sl],
                           False, True, 0)
                        mm(ps_d[:, :], ones_sb[0:1, 0:bl],
                           gb_d_sb[0:1, sl], False, True, 1)
                    a_sb = acts.tile([PB, 512], FP, tag=f"act{n}")
                    func = AF.Tanh if n == 2 else AF.Sigmoid
                    nc.scalar.activation(a_sb[0:bl, :], ps_e[:, :], func)
                    nc.scalar.activation(a_sb[DO:DO + bl, :], ps_d[:, :], func)
                    gate_sb[n] = a_sb

                t2 = elem.tile([PB, H], FP, tag="t2")
                nc.vector.tensor_mul(t2[:], gate_sb[0][:], gate_sb[2][:])  # i'*g'
                t1 = elem.tile([PB, H], FP, tag="t1")
                nc.vector.tensor_mul(t1[:], gate_sb[1][:], c_sb[:])        # f'*c
                nc.vector.tensor_add(c_sb[:], t1[:], t2[:])
                tnc = elem.tile([PB, H], FP, tag="tanh_c")
                nc.scalar.activation(tnc[:], c_sb[:], AF.Tanh)
                h_sb = elem.tile([PB, H], FR, tag="h")
                nc.vector.tensor_mul(h_sb[:], gate_sb[3][:], tnc[:])       # o'*tanh(c)

                nc.sync.dma_start(encP[:, t, :], h_sb[0:bl, :])
                nc.sync.dma_start(decP[:, t, :], h_sb[DO:DO + bl, :])

                hT_p = hTps.tile([128, KH * PB], FR, tag="hTp")
                for k in range(KH):
                    nc.tensor.transpose(
                        hT_p[:, PB * k:PB * (k + 1)],
                        h_sb[:, 128 * k:128 * (k + 1)], ident_sb[0:PB, 0:PB])
                nc.vector.tensor_copy(hT_sb[:], hT_p[:])

        # ---------------- phase 2: attention + output heads --------------------
        with (
            tc.tile_pool(name="pin", bufs=3) as pin,
            tc.tile_pool(name="ptmp", bufs=3) as ptmp,
            tc.tile_pool(name="psm", bufs=3) as psm,
            tc.tile_pool(name="tr_ps", bufs=1, space="PSUM") as trps,
            tc.tile_pool(name="s_ps", bufs=1, space="PSUM") as sps,
            tc.tile_pool(name="at_ps", bufs=1, space="PSUM") as atps,
            tc.tile_pool(name="cx_ps", bufs=1, space="PSUM") as cxps,
            tc.tile_pool(name="l_ps", bufs=1, space="PSUM") as lps,
            tc.tile_pool(name="r_ps", bufs=1, space="PSUM") as rps,
        ):
            for b in range(bl):
                encp_t = pin.tile([TS, H], FR, tag="encp")
                nc.sync.dma_start(encp_t[:], encP[b, :, :])
                decp_t = pin.tile([TS, H], FR, tag="decp")
                nc.sync.dma_start(decp_t[:], decP[b, :, :])

                trd = trps.tile([128, KH * TS], FR, tag="trd")
                for k in range(KH):
                    nc.tensor.transpose(
                        trd[:, TS * k:TS * (k + 1)],
                        decp_t[:, 128 * k:128 * (k + 1)], ident_sb[0:TS, 0:TS])
                decT = ptmp.tile([128, KH * TS], FR, tag="decT")
                nc.vector.tensor_copy(decT[:], trd[:])

                tre = trps.tile([128, KH * TS], FR, tag="tre")
                for k in range(KH):
                    nc.tensor.transpose(
                        tre[:, TS * k:TS * (k + 1)],
                        encp_t[:, 128 * k:128 * (k + 1)], ident_sb[0:TS, 0:TS])
                encT = ptmp.tile([128, KH * TS], FR, tag="encT")
                nc.scalar.copy(encT[:], tre[:])

                s_ps = sps.tile([TS, TS], FP, tag="s")
                for k in range(KH):
                    nc.tensor.matmul(
                        s_ps[:], decT[:, TS * k:TS * (k + 1)],
                        encT[:, TS * k:TS * (k + 1)],
                        start=(k == 0), stop=(k == KH - 1))

                negmax = psm.tile([TS, 1], FP, tag="negmax")
                nc.vector.reduce_max(
                    negmax[:], s_ps[:], axis=mybir.AxisListType.X, negate=True)
                esb = psm.tile([TS, TS], FP, tag="esb")
                rowsum = psm.tile([TS, 1], FP, tag="rowsum")
                nc.scalar.activation(
                    esb[:], s_ps[:], AF.Exp, bias=negmax[:], scale=1.0,
                    accum_out=rowsum[:])
                recip = psm.tile([TS, 1], FP, tag="recip")
                nc.vector.reciprocal(recip[:], rowsum[:])
                attn = psm.tile([TS, TS], FR, tag="attn")
                nc.vector.tensor_scalar_mul(attn[:], esb[:], recip[:])

                at_ps = atps.tile([TS, TS], FR, tag="at")
                nc.tensor.transpose(at_ps[:], attn[:], ident_sb[0:TS, 0:TS])
                atT = psm.tile([TS, TS], FR, tag="atT")
                nc.vector.tensor_copy(atT[:], at_ps[:])

                cx_ps = cxps.tile([128, KH * TS], FP, tag="cx")
                for m in range(KH):
                    nc.tensor.matmul(
                        cx_ps[:, TS * m:TS * (m + 1)],
                        encp_t[:, 128 * m:128 * (m + 1)], atT[:],
                        start=True, stop=True)
                ctxT = ptmp.tile([128, KH * TS], FR, tag="ctxT")
                nc.scalar.copy(ctxT[:], cx_ps[:])

                l_ps = lps.tile([TS, TS], FP, tag="l")
                for k in range(KH):
                    nc.tensor.matmul(
                        l_ps[:], ctxT[:, TS * k:TS * (k + 1)],
                        wd_sb[:, k, :], start=(k == 0),
                        stop=(k == KH - 1) and not out_bias)
                if out_bias:
                    nc.tensor.matmul(l_ps[:], ones_sb[0:1, 0:TS],
                                     bd_sb[0:1, :], start=False, stop=True)
                l_sb = psm.tile([TS, TS], FP, tag="l_sb")
                nc.scalar.copy(l_sb[:], l_ps[:])
                nc.sync.dma_start(out_logits[b, :, :], l_sb[:])

                r_ps = rps.tile([TS, 4], FP, tag="r")
                for k in range(KH):
                    nc.tensor.matmul(
                        r_ps[:], ctxT[:, TS * k:TS * (k + 1)],
                        wr_sb[:, k, :], start=(k == 0),
                        stop=(k == KH - 1) and not out_bias)
                if out_bias:
                    nc.tensor.matmul(r_ps[:], ones_sb[0:1, 0:TS],
                                     br_sb[0:1, :], start=False, stop=True)
                r_sb = psm.tile([TS, 4], FP, tag="r_sb")
                nc.vector.tensor_copy(r_sb[:], r_ps[:])
                nc.sync.dma_start(out_rot[b, :, :], r_sb[:])

    return _split_sync_waits(nc)


def pack_x(xs, t_steps, bl):
    # [bl, t, D] -> [128, KD, t, bl] with d = k*128 + p
    return np.ascontiguousarray(
        xs.transpose(2, 1, 0).reshape(KD, 128, t_steps, bl).transpose(1, 0, 2, 3)
    ).astype(np.float32)


def pack_w(w, kt):
    # [M, K] weight -> rhs-stream layout [128, kt, M] with k = ki*128 + p
    w = np.asarray(w)
    wT = np.ascontiguousarray(w.T)  # [K, M]
    return np.ascontiguousarray(
        wT.reshape(kt, 128, w.shape[0]).transpose(1, 0, 2)
    ).astype(np.float32)


def _get_nc(key):
    if key not in _BUILD_CACHE:
        gate_bias, out_bias = key
        _BUILD_CACHE[key] = build_nc(gate_bias=gate_bias, out_bias=out_bias)
    return _BUILD_CACHE[key]


def kernel(inputs, target_indices, enc_Wih, enc_Whh, enc_bih, enc_bhh,
           dec_Wih, dec_Whh, dec_bih, dec_bhh, Wd, bd, Wr, br):
    x = np.asarray(inputs, dtype=np.float32)
    idx = np.asarray(target_indices)
    idxc = np.clip(idx.astype(np.int64), 0, T - 1)

    # decoder inputs: x_dec[:, 0] = 0, x_dec[:, t] = x[b, idx[b, t-1]]
    xdec = np.zeros_like(x)
    xdec[:, 1:, :] = np.take_along_axis(x, idxc[:, :-1, None], axis=1)

    gbias_e = (np.asarray(enc_bih) + np.asarray(enc_bhh)).astype(np.float32)
    gbias_d = (np.asarray(dec_bih) + np.asarray(dec_bhh)).astype(np.float32)
    bd_np = np.asarray(bd, dtype=np.float32)
    br_np = np.asarray(br, dtype=np.float32)
    gate_bias = bool(np.any(gbias_e) or np.any(gbias_d))
    out_bias = bool(np.any(bd_np) or np.any(br_np))

    nc = _get_nc((gate_bias, out_bias))

    shared = {
        "wih_enc": pack_w(enc_Wih, KD),
        "whh_enc": pack_w(enc_Whh, KH),
        "wih_dec": pack_w(dec_Wih, KD),
        "whh_dec": pack_w(dec_Whh, KH),
        "wdT": pack_w(Wd, KH),
        "wrT": pack_w(Wr, KH),
        "ident": np.eye(128, dtype=np.float32),
    }
    if gate_bias:
        shared["gbias_enc"] = gbias_e[None, :]
        shared["gbias_dec"] = gbias_d[None, :]
    if out_bias:
        shared["bd"] = bd_np[None, :]
        shared["br"] = br_np[None, :]

    in_maps = []
    for c in range(N_CORES):
        b0 = c * BL
        m = dict(shared)
        m["xt_enc"] = pack_x(x[b0:b0 + BL], T, BL)
        m["xt_dec"] = pack_x(xdec[b0:b0 + BL], T, BL)
        in_maps.append(m)

    res = run_bass_kernel_spmd(
        nc, in_maps, list(range(N_CORES)), trace=TRACE,
        tmpdir=TRACE_DIR if TRACE else None)
    global LAST_EXEC_NS
    LAST_EXEC_NS = res.exec_time_ns
    logits = np.concatenate(
        [res.results[c]["logits"] for c in range(N_CORES)], axis=0)
    rot = np.concatenate(
        [res.results[c]["rot"] for c in range(N_CORES)], axis=0)

    # -inf mask: position l masked at step t if l in {idx[0..t-1]}
    onehot = np.zeros((B, T, L), dtype=bool)
    onehot[np.arange(B)[:, None], np.arange(T)[None, :], idxc] = True
    mask = np.zeros((B, T, L), dtype=bool)
    mask[:, 1:, :] = np.cumsum(onehot[:, :-1, :], axis=1) > 0
    logits[mask] = -np.inf
    return logits, rot
